# revision 45
# baseline (speedup 1.0000x reference)
"""Trainium2 Bass kernel for nn_DynamicConvolution.

Reference computation (per batch b, T=4096 timesteps, C=512 channels):
    h  = x @ w_in.T + b_in                    # (T, 2C)
    xg = h[:, :C] * sigmoid(h[:, C:])         # GLU -> (T, C)
    w  = softmax((xg @ w_wt.T + b_wt).reshape(T, H, K), axis=-1)
    out[c, t] = sum_k xg[t+k-3, c] * w[t, h(c), k]    # depthwise dynamic conv
    y  = (out + conv_bias) @ w_out.T + b_out

Sharding: data-parallel over batch B=8 -> one batch element per NeuronCore.

Two-phase structure per core, with all transposes on the xbar DMA engine
(no PE transposes) and a single resident ACT table:
  - phase 1: mm1 (x tiles stationary, bf16), GLU -> xg; one batched
    xbar DMA-transpose per 512-block -> xgT; dynamic-weight logits in
    C-major; softmax exp computed from the sigmoid table as
    e^x = sig(x)/sig(-x); exp tile DMA-transposed to token-major and
    normalized there (DVE k-reduce + reciprocal + broadcast mul);
    band weights shift-copied per 8-tile group (SBUF DMAs).
  - phase 2: per time tile, gpsimd local_scatter builds a 7-diagonal
    band matrix; banded matmuls (col-packed M=64 pairs) -> conv;
    cross-tile halos via DVE edge adds; mm_out contracts C; y staged
    bf16 in 4-tile slabs.
Inputs x and outputs y travel as bf16 (host casts).
"""

import os
import sys

import numpy as np

for _p in ("/opt/trn_rl_repo", os.path.expanduser("~/.axon_site/_ro/trn_rl_repo")):
    if os.path.isdir(_p) and _p not in sys.path:
        sys.path.insert(0, _p)

import concourse.bacc as bacc
import concourse.bass as bass
import concourse.mybir as mybir
import concourse.tile as tile
from concourse.bass_utils import run_bass_kernel_spmd

try:
    import ml_dtypes

    BF16 = np.dtype(ml_dtypes.bfloat16)
except ImportError:  # pragma: no cover
    BF16 = None

T, B, C = 4096, 8, 512
H, K = 8, 7
PAD_L = K // 2
C2 = 2 * C
HK = H * K  # 56
P = 128

F32 = mybir.dt.float32
BF = mybir.dt.bfloat16
I16 = mybir.dt.int16

MAIN_W = 136
DT_W = H * MAIN_W  # 1088
CW = P + 2 * PAD_L  # 134
GROUP = 8

SIG = mybir.ActivationFunctionType.Sigmoid


def ts(i, size):
    return slice(i * size, (i + 1) * size)


def host_scatter_idxs():
    """data[p, i*8+h] = wsm[t0+p+i-3, 7h+6-i] -> band column 136h + p + i."""
    p = np.arange(P)[:, None, None]
    i = np.arange(K)[None, :, None]
    h = np.arange(H)[None, None, :]
    idx = MAIN_W * h + p + i
    return np.ascontiguousarray(idx.reshape(P, K * H).astype(np.int16))


def build_nc(t_len=T, with_bias_in=False, with_bias_wt=False, with_bias_out=False,
             with_conv_bias=False):
    NT = t_len // P
    NTB = t_len // 512
    NG = (NT + GROUP - 1) // GROUP

    nc = bacc.Bacc()

    from contextlib import ExitStack
    _stack = ExitStack()

    x_d = nc.declare_dram_parameter("xT", [C, t_len], BF, isOutput=False)
    w_inT_d = nc.declare_dram_parameter("w_inT", [P, 4, C2], BF, isOutput=False)
    w_wtT_d = nc.declare_dram_parameter("w_wtT", [P, 4, HK], BF, isOutput=False)
    w_outT_d = nc.declare_dram_parameter("w_outT", [P, 4, C], BF, isOutput=False)
    idxs_d = nc.declare_dram_parameter("idxs", [P, HK], I16, isOutput=False)
    if with_bias_in:
        b_in_d = nc.declare_dram_parameter("b_in", [C2], F32, isOutput=False)
    if with_bias_wt:
        b_wt_d = nc.declare_dram_parameter("b_wt", [HK], F32, isOutput=False)
    if with_bias_out:
        b_out_d = nc.declare_dram_parameter("b_out", [C], F32, isOutput=False)
    if with_conv_bias:
        cb4_d = nc.declare_dram_parameter("cb4", [P, 4], F32, isOutput=False)
    y_d = nc.declare_dram_parameter("y", [t_len, C], BF, isOutput=True)
    y_v = y_d.rearrange("(b mm p) c -> b p mm c", mm=4, p=P)
    x_v = x_d.rearrange("(q p) t -> p q t", p=P)

    with tile.TileContext(nc) as tc:
        with (
            tc.tile_pool(name="const", bufs=1) as const,
            tc.tile_pool(name="big", bufs=1) as big,
            tc.tile_pool(name="work", bufs=3) as work,
            tc.tile_pool(name="dtp", bufs=3) as dtp,
        ):
            # ---- loads: x block 0 and w_inT first ----
            xT = big.tile([P, 4, t_len], BF)
            sb_winT = const.tile([P, 4, C2], BF)
            nc.sync.dma_start(xT[:, :, ts(0, 512)], x_v[:, :, ts(0, 512)])
            for q in range(4):
                nc.sync.dma_start(sb_winT[:, q, :], w_inT_d[:, q, :])
            for m4 in range(1, NTB):
                nc.sync.dma_start(xT[:, :, ts(m4, 512)], x_v[:, :, ts(m4, 512)])
            sb_wwtT = const.tile([P, 4, HK], BF)
            nc.gpsimd.dma_start(sb_wwtT[:], w_wtT_d[:])
            sb_woutT = const.tile([P, 4, C], BF)
            nc.gpsimd.dma_start(sb_woutT[:], w_outT_d[:])
            sb_idxs = const.tile([P, HK], I16)
            nc.gpsimd.dma_start(sb_idxs[:], idxs_d[:])
            if with_bias_in:
                sb_bin = const.tile([P, C2], F32)
                nc.sync.dma_start(sb_bin[:], b_in_d[None, :].to_broadcast((P, C2)))
            if with_bias_wt:
                sb_bwt = const.tile([HK, 1], F32)
                nc.sync.dma_start(sb_bwt[:], b_wt_d[:, None])
                sb_bwtn = const.tile([HK, 1], F32)
                nc.vector.tensor_scalar_mul(sb_bwtn[:], sb_bwt[:], -1.0)
            if with_bias_out:
                sb_bout = const.tile([P, C], F32)
                nc.sync.dma_start(sb_bout[:], b_out_d[None, :].to_broadcast((P, C)))
            if with_conv_bias:
                sb_cb4 = const.tile([P, 4], F32)
                nc.sync.dma_start(sb_cb4[:], cb4_d[:])

            # warm the sigmoid ACT table during the load window
            warm = const.tile([1, 2], F32)
            nc.scalar.activation(warm[:], sb_winT[0:1, 0, 0:2], SIG)

            # ---- persistent activations ----
            xg = big.tile([P, NT, C], BF)          # [t%128, t//128, c]
            xgT = big.tile([P, NT, 4, P], BF)      # [c%128, t//128, c//128, t%128]
            conv = big.tile([P, 4, t_len], BF)     # [c%128, c//128, t]
            wsm3 = big.tile([P, K, NT, H], BF)     # [t%128, k, t//128, h]
            data_tmp = big.tile([P, K, NT, H], BF)
            data_all = big.tile([P, NT, HK], BF)

            nc.gpsimd.memset(data_tmp[:], 0.0)

            e2c_bufs = []
            for _ in range(2):
                e2c = work.tile([64, 512], BF, tag="e2c")
                nc.gpsimd.memset(e2c[:], 0.0)
                e2c_bufs.append(e2c)

            # ---- phase-1 PSUM ----
            ps_mm1 = _stack.enter_context(
                tc.tile_pool(name="ps_mm1", bufs=3, space=bass.MemorySpace.PSUM))
            ps_w = _stack.enter_context(
                tc.tile_pool(name="ps_w", bufs=2, space=bass.MemorySpace.PSUM))

            def pass1b_tile(m):
                ps_a = ps_mm1.tile([P, C], F32, tag="ps_a")
                ps_g = ps_mm1.tile([P, C], F32, tag="ps_g")
                for q in range(4):
                    lhs = xT[:, q, ts(m, P)]
                    nc.tensor.matmul(ps_a[:], lhs, sb_winT[:, q, 0:C],
                                     start=(q == 0), stop=(q == 3))
                    nc.tensor.matmul(ps_g[:], lhs, sb_winT[:, q, C:C2],
                                     start=(q == 0), stop=(q == 3))
                sig = work.tile([P, C], F32, tag="sig")
                if with_bias_in:
                    tmp_g = work.tile([P, C], F32, tag="tmp_g")
                    nc.vector.tensor_add(tmp_g[:], ps_g[:], sb_bin[:, C:C2])
                    nc.scalar.activation(sig[:], tmp_g[:], SIG)
                    tmp_a = work.tile([P, C], F32, tag="tmp_a")
                    nc.vector.tensor_add(tmp_a[:], ps_a[:], sb_bin[:, 0:C])
                    nc.vector.tensor_mul(xg[:, m, :], tmp_a[:], sig[:])
                else:
                    nc.scalar.activation(sig[:], ps_g[:], SIG)
                    nc.vector.tensor_mul(xg[:, m, :], ps_a[:], sig[:])

            def xgT_block(n):
                # one batched xbar DMA-transpose for 4 tiles
                dst = xgT[:, 4 * n:4 * n + 4, :, :].rearrange(
                    "p mp q f -> p (mp q) f")
                src = xg[:, 4 * n:4 * n + 4, :].rearrange("p m c -> p (m c)")
                nc.sync.dma_start_transpose(dst, src)

            def pass1c_block(n):
                pw2 = ps_w.tile([HK, 512], F32, tag="pw2")
                for q in range(4):
                    nc.tensor.matmul(pw2[:], sb_wwtT[:, q, :],
                                     xgT[:, 4 * n:4 * n + 4, q, :],
                                     start=(q == 0), stop=(q == 3))
                # exp via the sigmoid table: e^x = sig(x) / sig(-x)
                sp = work.tile([HK, 512], F32, tag="sp")
                sn = work.tile([HK, 512], F32, tag="sn")
                if with_bias_wt:
                    nc.scalar.activation(sp[:], pw2[:], SIG, bias=sb_bwt[:])
                    nc.scalar.activation(sn[:], pw2[:], SIG, scale=-1.0,
                                         bias=sb_bwtn[:])
                else:
                    nc.scalar.activation(sp[:], pw2[:], SIG)
                    nc.scalar.activation(sn[:], pw2[:], SIG, scale=-1.0)
                rn = work.tile([HK, 512], F32, tag="rn")
                nc.vector.reciprocal_approx_fast(rn[:], sn[:])
                e2c = e2c_bufs[n % 2]
                with nc.allow_low_precision(reason="softmax exp in bf16"):
                    nc.vector.tensor_mul(e2c[0:HK, :], sp[:], rn[:])
                ptk = work.tile([P, 4, 64], BF, tag="ptk")
                nc.sync.dma_start_transpose(ptk[:], e2c[:])
                et = ptk[:, :, 0:HK].rearrange("p j (h k) -> p j h k", k=K)
                stok = work.tile([P, 4, H], F32, tag="stok")
                nc.vector.tensor_reduce(stok[:], et, mybir.AxisListType.X,
                                        mybir.AluOpType.add)
                rtok = work.tile([P, 4, H], F32, tag="rtok")
                nc.vector.reciprocal_approx_fast(rtok[:], stok[:])
                w_dst = wsm3[:, :, ts(n, 4), :].transpose([0, 2, 3, 1])
                with nc.allow_low_precision(reason="softmax weights in bf16"):
                    nc.vector.tensor_mul(
                        w_dst, et, rtok[:, :, :, None].to_broadcast((P, 4, H, K)))

            def build_group(g):
                mlo, mhi = g * GROUP, min((g + 1) * GROUP, NT)
                for i in range(K):
                    d = i - 3
                    kk = 6 - i
                    if d == 0:
                        nc.sync.dma_start(data_tmp[:, i, mlo:mhi, :],
                                          wsm3[:, kk, mlo:mhi, :])
                    elif d < 0:
                        nc.sync.dma_start(data_tmp[-d:P, i, mlo:mhi, :],
                                          wsm3[0:P + d, kk, mlo:mhi, :])
                        lo = max(mlo, 1)
                        if lo < mhi:
                            nc.gpsimd.dma_start(data_tmp[0:-d, i, lo:mhi, :],
                                                wsm3[P + d:P, kk, lo - 1:mhi - 1, :])
                    else:
                        nc.sync.dma_start(data_tmp[0:P - d, i, mlo:mhi, :],
                                          wsm3[d:P, kk, mlo:mhi, :])
                        hi = min(mhi, NT - 1)
                        if mlo < hi:
                            nc.gpsimd.dma_start(data_tmp[P - d:P, i, mlo:hi, :],
                                                wsm3[0:d, kk, mlo + 1:hi + 1, :])
                da4 = data_all[:, mlo:mhi, :].rearrange("p m (i h) -> p m i h", h=H)
                nc.vector.tensor_copy(
                    da4, data_tmp[:, :, mlo:mhi, :].transpose([0, 2, 1, 3]))

            dts = {}

            def scatter_tile(m):
                dt = dtp.tile([P, DT_W], BF, tag="dt")
                nc.gpsimd.local_scatter(dt[:], data_all[:, m, :], sb_idxs[:],
                                        channels=P, num_elems=DT_W, num_idxs=HK)
                dts[m] = dt

            # ---- phase-1 emission: pass1c lagged one 512-block so each
            # batched transpose has a block of slack ----
            p1c_done = 0
            for g in range(NG):
                for mi in range(g * GROUP, min((g + 1) * GROUP, NT)):
                    pass1b_tile(mi)
                    if mi % 4 == 3:
                        xgT_block(mi // 4)
                while p1c_done < min(2 * g + 1, NTB):
                    pass1c_block(p1c_done)
                    p1c_done += 1
                if g >= 1:
                    build_group(g - 1)
            # first band scatters go out ahead of the last group's
            # spill DMAs on the gpsimd queue, so phase 2 starts instantly
            scatter_tile(0)
            scatter_tile(1)
            while p1c_done < NTB:
                pass1c_block(p1c_done)
                p1c_done += 1
            build_group(NG - 1)

            # ---- phase 2 ----
            _stack.close()
            ps_c = _stack.enter_context(
                tc.tile_pool(name="ps_c", bufs=3, space=bass.MemorySpace.PSUM))
            ps_o = _stack.enter_context(
                tc.tile_pool(name="ps_o", bufs=2, space=bass.MemorySpace.PSUM))

            def conv_matmuls(m):
                dt = dts.pop(m)
                pc = ps_c.tile([P, 4, 256], F32, tag="pc")
                pc = pc[:, :, 0:CW]
                for ci in range(4):
                    for hp, pb in ((0, 0), (1, 64)):
                        hh = ci * 2 + hp
                        nc.tensor.matmul(
                            pc[pb:pb + 64, ci, :], xg[:, m, ts(hh, 64)],
                            dt[:, MAIN_W * hh:MAIN_W * hh + CW],
                            start=True, stop=True, skip_group_check=True)
                return pc

            state = {"out4": None}

            def mm_out(m):
                po = ps_o.tile([P, C], F32, tag="po")
                for q in range(4):
                    nc.tensor.matmul(po[:], conv[:, q, ts(m, P)], sb_woutT[:, q, :],
                                     start=(q == 0), stop=(q == 3))
                if m % 4 == 0:
                    state["out4"] = work.tile([P, 4, C], BF, tag="out4",
                                              name="out4", bufs=3)
                out4 = state["out4"]
                with nc.allow_low_precision(reason="y in bf16"):
                    if with_bias_out:
                        nc.vector.tensor_add(out4[:, m % 4, :], po[:], sb_bout[:])
                    else:
                        nc.vector.tensor_copy(out4[:, m % 4, :], po[:])
                if m == NT - 2:
                    nc.sync.dma_start(y_v[m // 4][:, 0:3, :], out4[:, 0:3, :])
                elif m == NT - 1:
                    nc.sync.dma_start(y_v[m // 4][:, 3:4, :], out4[:, 3:4, :])
                elif m % 4 == 3:
                    nc.sync.dma_start(y_v[m // 4], out4[:])

            el_prev = None
            for m in range(NT):
                if m + 2 < NT:
                    scatter_tile(m + 2)
                pc_m = conv_matmuls(m)
                # mm_out lags two tiles so its DVE/ACT feed chain (body
                # copy + edge adds of tile m-2) has a full tile of slack
                if m >= 2:
                    mm_out(m - 2)
                t0 = m * P
                if with_conv_bias:
                    for ci in range(4):
                        nc.vector.tensor_scalar_add(
                            conv[:, ci, t0:t0 + P], pc_m[:, ci, PAD_L:PAD_L + P],
                            sb_cb4[:, ci:ci + 1])
                else:
                    nc.scalar.copy(conv[:, :, t0:t0 + P],
                                   pc_m[:, :, PAD_L:PAD_L + P])
                if el_prev is not None:
                    dl = conv[:, :, t0:t0 + PAD_L]
                    nc.vector.tensor_add(dl, dl, el_prev[:])
                    dr = conv[:, :, t0 - PAD_L:t0]
                    nc.vector.tensor_add(dr, dr, pc_m[:, :, 0:PAD_L])
                if m + 1 < NT:
                    el = work.tile([P, 4, PAD_L], F32, tag="el")
                    nc.vector.tensor_copy(el[:], pc_m[:, :, CW - PAD_L:CW])
                    el_prev = el
            mm_out(NT - 2)
            mm_out(NT - 1)

            _stack.close()

    nc.compile()
    return nc


def host_inputs(x_b, w_in, b_in, w_wt, b_wt, w_out, b_out, conv_bias,
                with_bias_in, with_bias_wt, with_bias_out, with_conv_bias):
    def t_pack(w, width):
        return np.ascontiguousarray(
            w.T.reshape(4, P, width).transpose(1, 0, 2)).astype(BF16)

    m = {
        "xT": np.ascontiguousarray(np.asarray(x_b, np.float32).T).astype(BF16),
        "w_inT": t_pack(w_in, C2),
        "w_wtT": t_pack(w_wt, HK),
        "w_outT": t_pack(w_out, C),
        "idxs": host_scatter_idxs(),
    }
    if with_bias_in:
        m["b_in"] = np.asarray(b_in, np.float32)
    if with_bias_wt:
        m["b_wt"] = np.asarray(b_wt, np.float32)
    if with_bias_out:
        m["b_out"] = np.asarray(b_out, np.float32)
    if with_conv_bias:
        m["cb4"] = np.ascontiguousarray(
            np.asarray(conv_bias, np.float32).reshape(4, P).T)
    return m


_NC_CACHE = {}


def _get_nc(key):
    if key not in _NC_CACHE:
        _NC_CACHE[key] = build_nc(T, *key)
    return _NC_CACHE[key]


def kernel(x, w_in, b_in, w_wt, b_wt, w_out, b_out, conv_bias, _trace=False):
    x = np.asarray(x)
    flags = (bool(np.any(b_in)), bool(np.any(b_wt)), bool(np.any(b_out)),
             bool(np.any(conv_bias)))
    nc = _get_nc(flags)
    in_maps = [
        host_inputs(x[:, b, :], np.asarray(w_in), b_in, np.asarray(w_wt), b_wt,
                    np.asarray(w_out), b_out, conv_bias, *flags)
        for b in range(B)
    ]
    res = run_bass_kernel_spmd(nc, in_maps, core_ids=list(range(B)),
                               trace=_trace)
    y = np.stack([np.asarray(res.results[b]["y"]).astype(np.float32)
                  for b in range(B)], axis=1)
    if _trace:
        return y, res
    return y
